# revision 1
# baseline (speedup 1.0000x reference)
"""Distributed AttentionHead kernel for 8 TRN2 NeuronCores.

Problem: qkv = x @ w.T ; q,k,v = split(qkv); scores[i,j] = k_i.q_j/sqrt(E),
mask keeps j >= i; out = softmax(scores) @ v.   B,N,H,E = 4,2048,1024,1024.

Sharding: core c = 2*b + s handles batch b; it owns the 8 row-tiles
{128*(2*lt+s) : lt in 0..7} (parity interleave => every core's attention
loop has j-extents (16,14,12,...,2) tiles => a single uniform SPMD graph).
Masks that differ between even/odd cores are passed as input *data*,
keeping the graph identical on all cores (collectives-free SPMD).

Algebraic restructure (saves ~2x projection FLOPs vs materializing q/k/v):
  scores = X (Wk^T Wq) X^T = X M X^T    -> M first (weights only!), then
                                           T = X_own M, S = T X^T
  out    = P (X Wv^T)   = (P X) Wv^T    -> U = P X, own rows only
M costs the same 1.07 GMAC as the K-projection it replaces, but it
depends only on the weights, so the PE starts ~4us earlier (first M
group needs 1.25 MB of weight DMA instead of 2.5 MB of wk+x^T).

Per-core work: M 2.15 GF + T 2.15 + scores ~2.4 + U ~2.4 + out 2.15
= ~11.3 GF.  All operands are staged to bf16 on the host (halves HBM
traffic vs f32+cast-in-DMA), fp32 accum; output is written bf16 and
widened on the host.
"""
import os
import sys

sys.path.insert(0, "/opt/trn_rl_repo")

import numpy as np
import ml_dtypes

import concourse.mybir as mybir
from concourse import bacc
from concourse.tile import TileContext
from concourse.bass_utils import run_bass_kernel_spmd

B, N, H, E = 4, 2048, 1024, 1024
NT = N // 128          # 16 row tiles per batch
LT = 8                 # row tiles owned per core
BF = mybir.dt.bfloat16
F32 = mybir.dt.float32

_CACHE = {}
LAST_RESULT = None


def _build():
    nc = bacc.Bacc("TRN2", target_bir_lowering=False, debug=False, num_devices=8)

    xT_ext = nc.dram_tensor("xT", [H, N], BF, kind="ExternalInput")
    xn_ext = nc.dram_tensor("xn", [N, H], BF, kind="ExternalInput")
    wq_ext = nc.dram_tensor("wQ", [E, H], BF, kind="ExternalInput")
    wk_ext = nc.dram_tensor("wk", [E, H], BF, kind="ExternalInput")
    wvT_ext = nc.dram_tensor("wvT", [H, E], BF, kind="ExternalInput")
    am_ext = nc.dram_tensor("amask", [128, 256], F32, kind="ExternalInput")
    id_ext = nc.dram_tensor("ident", [128, 128], BF, kind="ExternalInput")
    out_ext = nc.dram_tensor("out", [LT, 128, E], BF, kind="ExternalOutput")

    xT_r = xT_ext.rearrange("(hs p) n -> p hs n", p=128)
    xn_r = xn_ext.rearrange("(jt p) h -> p jt h", p=128)
    wq_r = wq_ext.rearrange("(oc p) h -> p oc h", p=128)
    wk_r = wk_ext.rearrange("(oc p) h -> p oc h", p=128)
    wvT_r = wvT_ext.rearrange("(hs p) e -> p hs e", p=128)

    with TileContext(nc) as tc:
        with (
            tc.tile_pool(name="consts", bufs=1) as consts,
            tc.tile_pool(name="wts", bufs=1) as wts,
            tc.tile_pool(name="bigx", bufs=1) as bigx,
            tc.tile_pool(name="qkv", bufs=1) as qkv,
            tc.tile_pool(name="pbuf", bufs=1) as pbuf,
            tc.tile_pool(name="pts", bufs=6) as ptsp,
            tc.tile_pool(name="ubuf", bufs=3) as ubuf,
            tc.tile_pool(name="utb", bufs=2) as utb,
            tc.tile_pool(name="outb", bufs=2) as outb,
            tc.tile_pool(name="smalls", bufs=3) as smalls,
            tc.tile_pool(name="acc", bufs=4, space="PSUM") as accp,
            tc.tile_pool(name="sc", bufs=2, space="PSUM") as scp,
            tc.tile_pool(name="tp", bufs=2, space="PSUM") as tpp,
        ):
            ident = consts.tile([128, 128], BF)
            nc.sync.dma_start(out=ident, in_=id_ext[:, :])
            am_sb = consts.tile([128, 256], F32)
            nc.sync.dma_start(out=am_sb, in_=am_ext[:, :])

            # Warm-up matmuls: keep the PE busy while the first weight
            # chunks load so the HAM clock gate ramps before real work.
            wu_sb = consts.tile([128, 512], BF)
            nc.vector.memset(wu_sb, 0.0)
            wu_lhs = consts.tile([128, 128], BF)
            nc.vector.memset(wu_lhs, 0.0)
            wu_ps = accp.tile([128, 512], F32, tag="acc", name="wu_ps")
            for r in range(8):
                nc.tensor.matmul(wu_ps, wu_lhs, wu_sb, start=True, stop=True)

            # M-phase inputs, ordered for earliest possible PE start: the
            # first M psum group (h-chunk 0, h' half 0) needs only wk's
            # first 128-column block and wq's first 512 columns.
            wk_sb = wts.tile([128, 8, H], BF, tag="wk", name="wk_sb")
            nc.gpsimd.dma_start(out=wk_sb[:, :, 0:128], in_=wk_r[:, :, 0:128])
            wq_sb = bigx.tile([128, 8, H], BF)
            nc.gpsimd.dma_start(out=wq_sb[:, :, 0:512], in_=wq_r[:, :, 0:512])
            for hc in range(1, 8):
                nc.gpsimd.dma_start(
                    out=wk_sb[:, :, 128 * hc:128 * hc + 128],
                    in_=wk_r[:, :, 128 * hc:128 * hc + 128],
                )
            nc.gpsimd.dma_start(out=wq_sb[:, :, 512:1024], in_=wq_r[:, :, 512:1024])

            xT_sb = bigx.tile([128, 8, N], BF)
            for hs in range(8):
                nc.gpsimd.dma_start(
                    out=xT_sb[:, hs, 0:N // 2], in_=xT_r[:, hs, 0:N // 2]
                )
            nc.gpsimd.dma_start(out=xT_sb[:, :, N // 2:N], in_=xT_r[:, :, N // 2:N])

            xn_sb = qkv.tile([128, NT, H], BF, tag="xn", name="xn_sb")
            nc.gpsimd.dma_start(out=xn_sb, in_=xn_r)
            xn = [xn_sb[:, t, :] for t in range(NT)]

            wvT_sb = bigx.tile([128, 8, E], BF)
            nc.gpsimd.dma_start(out=wvT_sb, in_=wvT_r)

            TT = [qkv.tile([128, N // 2], BF, tag=f"TT{h}", name=f"TT{h}") for h in range(8)]

            # ---------------- M = Wk^T Wq  [H, H] ----------------
            # M[h, h'] = sum_o Wk[o, h] Wq[o, h']; weights-only, so this
            # phase runs while x^T still streams in.
            m_sb = wts.tile([128, 8, H], BF, tag="m", name="m_sb")
            for half in range(2):
                for hc in range(8):
                    ps = accp.tile([128, 512], F32, tag="acc", name="ps_m")
                    for oc in range(8):
                        nc.tensor.matmul(
                            ps,
                            wk_sb[:, oc, 128 * hc:128 * hc + 128],
                            wq_sb[:, oc, 512 * half:512 * half + 512],
                            start=oc == 0,
                            stop=oc == 7,
                        )
                    nc.scalar.copy(
                        out=m_sb[:, hc, 512 * half:512 * half + 512], in_=ps
                    )

            # ---------------- T^T = (X_own M)^T = M^T x_own^T ----------------
            for i0 in range(0, N // 2, 512):
                for ht in range(8):
                    ps = accp.tile([128, 512], F32, tag="acc", name="ps_t")
                    for hs in range(8):
                        nc.tensor.matmul(
                            ps,
                            m_sb[:, hs, 128 * ht:128 * ht + 128],
                            xT_sb[:, hs, i0:i0 + 512],
                            start=hs == 0,
                            stop=hs == 7,
                        )
                    nc.vector.tensor_copy(out=TT[ht][:, i0:i0 + 512], in_=ps)

            # ---------------- attention ----------------
            # The out-projection of row-block li-1 is emitted between the
            # S-phase and U-phase of block li: its matmuls keep the PE fed
            # while exp(li) runs on the scalar engine.
            pending_out = None
            for li in range(LT):
                nch = 8 - li          # 256-wide score chunks
                nj = NT - 2 * li      # 128-wide j tiles
                p = pbuf.tile([128, 256 * nch], BF, tag=f"p{li}", name=f"p{li}")
                asum = smalls.tile([128, 8], F32, tag="asum", name=f"asum{li}")
                # 512-wide score chunks (two own/other tile-pairs per psum
                # group) halve the S accumulation-group count; the rhs dims
                # are ordered (g, two, c) so p keeps the same
                # [own g | other g | own g+1 | other g+1] tile order the
                # U-phase transposes index into.
                nch2 = (nch + 1) // 2
                for c2 in range(nch2):
                    g = li + 2 * c2
                    cw = 512 if 2 * c2 + 1 < nch else 256
                    ps = scp.tile([128, cw], F32, tag="sc", name=f"ps_s{li}_{c2}")
                    for hs in range(8):
                        if cw == 512:
                            rhs = xT_sb[:, hs, :].rearrange(
                                "p (two g c) -> p g two c", two=2, c=128
                            )[:, g:g + 2, :, :]
                        else:
                            rhs = xT_sb[:, hs, :].rearrange(
                                "p (two g c) -> p two g c", two=2, c=128
                            )[:, :, g, :]
                        nc.tensor.matmul(
                            ps,
                            TT[hs][:, 128 * li:128 * li + 128],
                            rhs,
                            start=hs == 0,
                            stop=hs == 7,
                        )
                    if c2 == 0:
                        nc.vector.tensor_add(ps[:, 0:256], ps[:, 0:256], am_sb)
                    nc.scalar.activation(
                        out=p[:, 512 * c2:512 * c2 + cw],
                        in_=ps,
                        func=mybir.ActivationFunctionType.Exp,
                        scale=float(1.0 / np.sqrt(E)),
                        accum_out=asum[:, c2:c2 + 1],
                    )
                den = smalls.tile([128, 1], F32, tag="den", name=f"den{li}")
                nc.vector.reduce_sum(den, asum[:, 0:nch2], axis=mybir.AxisListType.X)
                rden = smalls.tile([128, 1], F32, tag="rden", name=f"rden{li}")
                nc.vector.reciprocal(rden, den)

                if pending_out is not None:
                    pending_out()
                    pending_out = None

                ut = utb.tile([128, 8, 128], BF, tag="ut", name=f"ut{li}")
                if li < 5:
                    # U = P X  (f32 accum in PSUM, bf16 out), then transpose U
                    pv0 = accp.tile([128, 512], F32, tag="acc", name=f"pv0_{li}")
                    pv1 = accp.tile([128, 512], F32, tag="acc", name=f"pv1_{li}")
                    for u in range(nj):
                        tp = tpp.tile([128, 128], BF, tag="tp", name=f"tp{li}_{u}")
                        nc.tensor.transpose(tp, p[:, 128 * u:128 * u + 128], ident)
                        pt = ptsp.tile([128, 128], BF, tag="pts", name=f"pt{li}_{u}")
                        nc.vector.tensor_copy(out=pt, in_=tp)
                        jt = (li + u // 2) + (8 if u % 2 else 0)
                        nc.tensor.matmul(
                            pv0, pt, xn[jt][:, 0:512], start=u == 0, stop=u == nj - 1
                        )
                        nc.tensor.matmul(
                            pv1, pt, xn[jt][:, 512:1024], start=u == 0, stop=u == nj - 1
                        )
                    usb = ubuf.tile([128, H], BF, tag="u", name=f"u{li}")
                    nc.scalar.copy(out=usb[:, 0:512], in_=pv0)
                    nc.scalar.copy(out=usb[:, 512:1024], in_=pv1)
                    for hs in range(8):
                        tp = tpp.tile([128, 128], BF, tag="tp", name=f"tpu{li}_{hs}")
                        nc.tensor.transpose(tp, usb[:, 128 * hs:128 * hs + 128], ident)
                        nc.vector.tensor_copy(out=ut[:, hs, :], in_=tp)
                else:
                    # small j-window: accumulate U^T directly (shorter serial
                    # chain; PE has slack here)
                    pts_list = []
                    for u in range(nj):
                        tp = tpp.tile([128, 128], BF, tag="tp", name=f"tp{li}_{u}")
                        nc.tensor.transpose(tp, p[:, 128 * u:128 * u + 128], ident)
                        pt = ptsp.tile([128, 128], BF, tag="pts", name=f"pt{li}_{u}")
                        nc.vector.tensor_copy(out=pt, in_=tp)
                        pts_list.append(pt)
                    for ht in range(8):
                        up = accp.tile([128, 128], F32, tag="acc", name=f"up{li}_{ht}")
                        for u in range(nj):
                            jt = (li + u // 2) + (8 if u % 2 else 0)
                            nc.tensor.matmul(
                                up,
                                xn[jt][:, 128 * ht:128 * ht + 128],
                                pts_list[u],
                                start=u == 0,
                                stop=u == nj - 1,
                            )
                        nc.vector.tensor_copy(out=ut[:, ht, :], in_=up)

                # out = U Wv^T, then normalize by the softmax denominator.
                # cw: psum-group width; the last block uses 256 so the final
                # drain (vector mul + DMA) after the last matmul is shorter.
                def emit_out(li=li, ut=ut, rden=rden, cw=512):
                    ob = outb.tile([128, 1024], BF, tag="ob", name=f"ob{li}")
                    for e0 in range(0, 1024, cw):
                        pool, tg = (accp, "acc") if cw == 512 else (scp, "sc")
                        po = pool.tile([128, cw], F32, tag=tg, name=f"po{li}_{e0}")
                        for hs in range(8):
                            nc.tensor.matmul(
                                po,
                                ut[:, hs, :],
                                wvT_sb[:, hs, e0:e0 + cw],
                                start=hs == 0,
                                stop=hs == 7,
                            )
                        nc.vector.tensor_scalar_mul(ob[:, e0:e0 + cw], po, rden)
                        nc.sync.dma_start(
                            out=out_ext[li, :, e0:e0 + cw], in_=ob[:, e0:e0 + cw]
                        )

                pending_out = emit_out

            pending_out(cw=256)

    nc.compile()
    return nc


def _amask(s: int) -> np.ndarray:
    # Additive mask for chunk 0 = [own diagonal tile | partner tile]; the
    # partner tile of slot li is global tile 2li+(1-s): above the diagonal
    # for s=0 (keep), below for s=1 (mask out).
    m = np.zeros((128, 256), dtype=np.float32)
    i = np.arange(128)[:, None]
    j = np.arange(128)[None, :]
    m[:, 0:128] = np.where(j >= i, 0.0, -1e9).astype(np.float32)
    if s == 1:
        m[:, 128:256] = -1e9
    return m


def _perm(s: int) -> np.ndarray:
    own = [2 * u + s for u in range(8)]
    other = [2 * u + 1 - s for u in range(8)]
    return np.array(own + other)


def kernel(input: np.ndarray, w: np.ndarray) -> np.ndarray:
    global LAST_RESULT
    if "nc" not in _CACHE:
        _CACHE["nc"] = _build()
    nc = _CACHE["nc"]

    bf16 = ml_dtypes.bfloat16
    xb = np.asarray(input, dtype=np.float32).astype(bf16)       # [B, N, H]
    wb = np.asarray(w, dtype=np.float32).astype(bf16)           # [3E, H]
    wq = np.ascontiguousarray(wb[0:E, :])                       # [E, H]
    wk = np.ascontiguousarray(wb[E:2 * E, :])                   # [E, H]
    wvT = np.ascontiguousarray(wb[2 * E:3 * E, :].T)            # [H, E]
    ident = np.eye(128, dtype=bf16)

    in_maps = []
    for c in range(8):
        b, s = divmod(c, 2)
        perm = _perm(s)
        xt3 = xb[b].T.reshape(H, NT, 128)                       # [H, 16, 128]
        xT = np.ascontiguousarray(
            xt3[:, perm, :].reshape(H, N)
        )                                                       # [H, N] col-tiles permuted
        xn3 = xb[b].reshape(NT, 128, H)
        xn = np.ascontiguousarray(
            xn3[perm].reshape(N, H)
        )                                                       # [N, H] row-tiles permuted
        in_maps.append(
            {
                "xT": xT,
                "xn": xn,
                "wQ": wq,
                "wk": wk,
                "wvT": wvT,
                "amask": _amask(s),
                "ident": ident,
            }
        )

    trace = bool(int(os.environ.get("KERNEL_TRACE", "0")))
    res = run_bass_kernel_spmd(nc, in_maps, core_ids=list(range(8)), trace=trace)
    LAST_RESULT = res

    out = np.empty((B, N, E), dtype=np.float32)
    for c in range(8):
        b, s = divmod(c, 2)
        o = np.asarray(res.results[c]["out"], dtype=np.float32)  # [LT, 128, 1024]
        for lt in range(LT):
            r0 = 128 * (2 * lt + s)
            out[b, r0:r0 + 128, :] = o[lt]
    return out

